# revision 6
# baseline (speedup 1.0000x reference)
"""Trainium2 Bass kernel for nn_DecomposedConvolutionalAttention.

Strategy (data-parallel over batch, one sample per NeuronCore):
  host:   SE-MLP kernel prediction (tiny, f32-exact), builds per-channel
          13x13 kernels laid out as a Hankel "arena" so the device can
          materialize banded Toeplitz stationary tiles with one 3D DMA;
          ships x1 transposed/h-flipped as bf16.  x2 half is a passthrough.
  device: fused 1x1-conv+layout-pivot (image slices as matmul stationary),
          then the depthwise 13x13 conv as 13 PSUM-accumulated banded
          matmuls per (channel, h-window) on the tensor engine (bf16).
"""
import sys
import math
import functools

sys.path.insert(0, '/opt/trn_rl_repo')

import numpy as np
import ml_dtypes

PDIM, B, C, H, W = 64, 8, 128, 256, 256
LK, DK, PAD = 13, 3, 5
NCORES = 8
HPADDED = H + 12                      # 268
WIN_STARTS = [-6, 110, 226]           # device h_in window starts (128 rows each)
WIN_M = [(6, 122), (6, 122), (98, 122)]  # valid psum partition range per window
CG = 2                                # channels per depthwise group
AR = 255                              # arena row length


@functools.lru_cache(maxsize=2)
def _build_program(loops=1):
    import contextlib
    import dataclasses
    import concourse.bacc as bacc
    import concourse.mybir as mybir
    from concourse.tile import TileContext

    bf16 = mybir.dt.bfloat16
    f32 = mybir.dt.float32

    nc = bacc.Bacc("TRN2", target_bir_lowering=False, debug=False)

    xT = nc.dram_tensor("xT", [PDIM, W, HPADDED], bf16, kind="ExternalInput")
    arena = nc.dram_tensor("arena", [PDIM, LK, AR], bf16, kind="ExternalInput")
    wct = nc.dram_tensor("wct", [PDIM, PDIM], bf16, kind="ExternalInput")
    y1 = nc.dram_tensor("y1", [PDIM, H, W], f32, kind="ExternalOutput")

    with TileContext(nc) as tc:
        loop_cm = tc.For_i(0, loops, 1) if loops > 1 else contextlib.nullcontext()
        with loop_cm, (
            tc.tile_pool(name="xh", bufs=1)) as xh_pool, (
            tc.tile_pool(name="tt", bufs=3)) as tt_pool, (
            tc.tile_pool(name="osb", bufs=3)) as out_pool, (
            tc.tile_pool(name="wct", bufs=1)) as wct_pool:
            wct_sb = wct_pool.tile([PDIM, PDIM], bf16)
            nc.sync.dma_start(out=wct_sb[:, :], in_=wct[:, :])

            # ---- phase A: per-window load + fused 1x1-conv/pivot ----
            xh_tiles = []
            with (
                tc.tile_pool(name="x1w", bufs=1) as xw_pool,
                tc.tile_pool(name="ppiv", bufs=4, space="PSUM") as ppiv_pool,
            ):
                for t, ws in enumerate(WIN_STARTS):
                    hp0 = ws + 6
                    nrows = min(128, HPADDED - hp0)
                    x1win = xw_pool.tile([PDIM, W * 128], bf16)
                    x1r = x1win[:, :].rearrange("c (w p) -> c w p", p=128)
                    nc.sync.dma_start(out=x1r[:, :, :nrows],
                                      in_=xT[:, :, hp0:hp0 + nrows])
                    if nrows < 128:
                        nc.gpsimd.memset(x1r[:, :, nrows:], 0.0)

                    xh = xh_pool.tile([128, PDIM * HPADDED], bf16, tag=f"xh{t}")
                    xh_r = xh[:, :].rearrange("p (o w) -> p o w", w=HPADDED)
                    nc.gpsimd.memset(xh_r[:, :, 0:6], 0.0)
                    nc.gpsimd.memset(xh_r[:, :, 6 + W:], 0.0)
                    for w in range(W):
                        ps = ppiv_pool.tile([128, PDIM], f32)
                        nc.tensor.matmul(ps[:, :],
                                         lhsT=x1r[:, w, :],
                                         rhs=wct_sb[:, :],
                                         start=True, stop=True)
                        nc.vector.tensor_copy(xh_r[:, :, 6 + w], ps[:, :])
                    xh_tiles.append(xh_r)

            # ---- phase B: depthwise 13x13 via banded matmuls ----
            arena_ap = arena.ap()
            with tc.tile_pool(name="pdw", bufs=8, space="PSUM") as pdw_pool:
                for cg in range(PDIM // CG):
                    psums = {}
                    for j in range(CG):
                        for t in range(len(WIN_STARTS)):
                            psums[(j, t)] = pdw_pool.tile(
                                [128, W], f32, tag="pdw", name=f"pdw{j}_{t}")
                    for dx in range(LK):
                        tt = tt_pool.tile([128, CG * 128], bf16)
                        hank = dataclasses.replace(
                            arena_ap,
                            offset=(cg * CG) * (LK * AR) + dx * AR,
                            ap=[[1, 128], [LK * AR, CG], [1, 128]],
                        )
                        nc.gpsimd.dma_start(out=tt[:, :], in_=hank)
                        for j in range(CG):
                            for t in range(len(WIN_STARTS)):
                                c = cg * CG + j
                                nc.tensor.matmul(
                                    psums[(j, t)][:, :],
                                    lhsT=tt[:, j * 128:(j + 1) * 128],
                                    rhs=xh_tiles[t][:, c, dx:dx + W],
                                    start=(dx == 0), stop=(dx == LK - 1))
                    for j in range(CG):
                        c = cg * CG + j
                        for t, ws in enumerate(WIN_STARTS):
                            m_lo, m_hi = WIN_M[t]
                            n = m_hi - m_lo
                            o_sb = out_pool.tile([128, W], f32)
                            nc.vector.tensor_copy(o_sb[:, :],
                                                  psums[(j, t)][:, :])
                            r0 = (128 - ws) + m_lo
                            nc.sync.dma_start(out=y1[c, r0:r0 + n, :],
                                              in_=o_sb[m_lo:m_hi, :])
    nc.compile()
    return nc


def _host_precompute(x, lk_channel, lk_spatial, w1, b1, w2, b2):
    x = np.asarray(x)
    x1 = x[:, :PDIM]

    pooled = x1.mean(axis=(2, 3), dtype=np.float64).astype(np.float32)
    pre = pooled @ np.asarray(w1).T + np.asarray(b1)
    erf = np.vectorize(math.erf)
    hid = (pre * 0.5 * (1.0 + erf(pre / np.sqrt(2.0)))).astype(np.float32)
    kern = (hid @ np.asarray(w2).T + np.asarray(b2)).reshape(B, PDIM, DK, DK)
    kfull = np.zeros((B, PDIM, LK, LK), np.float32)
    kfull[:, :, PAD:PAD + DK, PAD:PAD + DK] = kern
    kfull += np.asarray(lk_spatial)[None, :, 0]

    arena = np.zeros((B, PDIM, LK, AR), ml_dtypes.bfloat16)
    arena[:, :, :, 121:121 + LK] = np.transpose(kfull[:, :, ::-1, :], (0, 1, 3, 2))

    xT = np.zeros((B, PDIM, W, HPADDED), ml_dtypes.bfloat16)
    xT[:, :, :, 6:6 + H] = np.transpose(x1[:, :, ::-1, :], (0, 1, 3, 2))

    wct = np.ascontiguousarray(
        np.asarray(lk_channel)[:, :, 0, 0].T).astype(ml_dtypes.bfloat16)
    return xT, arena, wct


def kernel(x, lk_channel, lk_spatial, w1, b1, w2, b2, _trace=False):
    from concourse.bass_utils import run_bass_kernel_spmd

    x = np.asarray(x)
    xT, arena, wct = _host_precompute(x, lk_channel, lk_spatial, w1, b1, w2, b2)

    nc = _build_program()
    in_maps = [
        {"xT": xT[b], "arena": arena[b], "wct": wct}
        for b in range(B)
    ]
    res = run_bass_kernel_spmd(nc, in_maps, core_ids=list(range(NCORES)),
                               trace=_trace)

    out = np.empty((B, C, H, W), np.float32)
    for b in range(B):
        out[b, :PDIM] = res.results[b]["y1"]
    out[:, PDIM:] = x[:, PDIM:]
    if _trace:
        kernel.last_exec_time_ns = res.exec_time_ns
    return out
